# revision 61
# baseline (speedup 1.0000x reference)
"""Multi-head attention (B=4, N=2048, DIM=1024, H=16, DH=64) on 8 trn2 cores.

Sharding: core c handles batch c//2 and head-half c%2 (8 heads).  Each core
computes qkv projection for its heads, attention, and a partial output
projection; the host sums the two partials per batch and adds the bias.
No cross-core collectives needed.

Layout strategy (zero on-device transposes):
  - host supplies x[b] pre-transposed (xT: [DIM, N]) in bf16
  - qT/kT computed as [d, n] ("transposed") via out = W^T @ x^T matmuls
  - S^T tiles [j=128, i=512] from row-packed matmuls (d=64 contraction,
    2 heads concurrently in PE row groups 0-63 / 64-127)
  - exp via ACT (scale folded), PSUM -> SBUF bf16 (P^T tiles)
  - PV: O^T[d, i] += V[j, d]^T-matmul, col-packed pairs (PE col groups),
    delayed two j-iterations behind S so the PE never waits on the
    current exp (breaks the per-iteration S->exp->PV semaphore stall)
  - denominators: batched strided-AP DVE add-tree over j-tiles, ones-matmul
    partition reduce, reciprocal straight off PSUM, gpsimd broadcast; the
    whole tail is deferred into the NEXT unit's j-loop so it never stalls
    the in-order PE queue at a unit boundary
  - first unit software-floods the V projection as per-iteration extras so
    exp starts ~27us earlier; input DMAs are issued across SP+ACT queues
    ordered by first use
  - output projection consumes O^T tiles directly as lhsT; the final
    i-block's first steps pre-accumulate pairs 0-2 to keep the PE warm
    while the last denominator drains
"""

import numpy as np
import ml_dtypes

B, N, DIM = 4, 2048, 1024
HEADS, DH = 16, 64
SCALE = DIM ** (-0.5)
HPC = 8              # heads per core
NPAIR = HPC // 2     # 4 head pairs
CPC = HPC * DH       # 512 channels per core
IB = 512             # i-block (query cols per attention unit)
NIB = N // IB        # 4
NJT = N // 128       # 16 j-tiles
NKT = DIM // 128     # 8 contraction tiles for projections

_cache = {}


def _build():
    import concourse.bacc as bacc
    import concourse.mybir as mybir
    import concourse.tile as tile

    f32 = mybir.dt.float32
    bf16 = mybir.dt.bfloat16

    nc = bacc.Bacc("TRN2", target_bir_lowering=False, debug=False,
                   enable_asserts=False, num_devices=8)

    xT_d = nc.dram_tensor("xT", (DIM, N), bf16, kind="ExternalInput").ap()
    wqkv_d = nc.dram_tensor("wqkv", (DIM, 3 * CPC), bf16, kind="ExternalInput").ap()
    wout_d = nc.dram_tensor("wout", (CPC, DIM), bf16, kind="ExternalInput").ap()
    out_d = nc.dram_tensor("out", (N, DIM), bf16, kind="ExternalOutput").ap()

    with tile.TileContext(nc) as tc:
        _body(nc, tc, mybir, xT_d, wqkv_d, wout_d, out_d)

    nc.compile()
    return nc


def _body(nc, tc, mybir, xT_d, wqkv_d, wout_d, out_d):
    import concourse.bass_isa as bass_isa
    from contextlib import ExitStack

    f32 = mybir.dt.float32
    bf16 = mybir.dt.bfloat16
    Exp = mybir.ActivationFunctionType.Exp
    mult = mybir.AluOpType.mult
    add = mybir.AluOpType.add
    NJH = NJT // 2   # j-tiles per half (8)

    ctx = ExitStack()
    with ctx:
        wpool = ctx.enter_context(tc.tile_pool(name="weights", bufs=1))
        qkv_pool = ctx.enter_context(tc.tile_pool(name="qkv", bufs=1))
        ppool = ctx.enter_context(tc.tile_pool(name="ptiles", bufs=2))
        ppool1 = ctx.enter_context(tc.tile_pool(name="ptiles1", bufs=1))
        spool = ctx.enter_context(tc.tile_pool(name="small", bufs=2))
        outp = ctx.enter_context(tc.tile_pool(name="outstage", bufs=3))
        opool = ctx.enter_context(tc.tile_pool(name="oT", bufs=16))
        psum = ctx.enter_context(tc.tile_pool(name="psum", bufs=2, space="PSUM"))

        # ---- weights + xT load, ordered by first use: k-cols + xT i-block 0
        # feed the prologue, then q-cols, v-cols, remaining xT i-blocks ----
        wqkv_sb = wpool.tile([128, NKT, 3 * CPC], bf16)
        wqkv_r = wqkv_d.rearrange("(ko p) c -> p ko c", p=128)
        xT_sb = wpool.tile([128, NKT, N], bf16)
        xT_r = xT_d.rearrange("(ko p) n -> p ko n", p=128)
        # one contiguous [0:640] chunk per k-tile covers all q columns plus
        # pair-0 k columns with descriptor-efficient 1.25KB rows
        for kt in range(NKT):
            nc.sync.dma_start(wqkv_sb[:, kt, 0:CPC + 128],
                              wqkv_r[:, kt, 0:CPC + 128])
            nc.scalar.dma_start(xT_sb[:, kt, 0:IB], xT_r[:, kt, 0:IB])
        nc.scalar.dma_start(wqkv_sb[:, :, 2 * CPC:3 * CPC],
                            wqkv_r[:, :, 2 * CPC:3 * CPC])
        nc.sync.dma_start(wqkv_sb[:, :, CPC + 128:2 * CPC],
                          wqkv_r[:, :, CPC + 128:2 * CPC])
        # xT i-blocks 1-3 ride the sync queue (which drains early) so the
        # flood's k/q chunk extras never wait on the scalar queue backlog
        for ib in range(1, NIB):
            nc.sync.dma_start(xT_sb[:, :, ib * IB:(ib + 1) * IB],
                              xT_r[:, :, ib * IB:(ib + 1) * IB])
        wout_sb = wpool.tile([128, NPAIR, DIM], bf16)
        nc.scalar.dma_start(wout_sb, wout_d.rearrange("(po p) n -> p po n", p=128))

        ones_sb = wpool.tile([128, 1], bf16)
        nc.gpsimd.memset(ones_sb, 1.0)

        # per-pair q/k tiles (separate tiles => clean dependency tracking
        # when later pairs' projections interleave into attention units)
        qT_t = [qkv_pool.tile([128, N], bf16, tag=f"qT{p}", name=f"qT{p}") for p in range(NPAIR)]
        kT_t = [qkv_pool.tile([128, N], bf16, tag=f"kT{p}", name=f"kT{p}") for p in range(NPAIR)]
        v_sb = qkv_pool.tile([128, NJT, CPC], bf16)

        # ---- emit helpers ----
        def qk_steps(p, k_first=False):
            """Projection of qT/kT for pair p as a list of small PE bursts."""
            steps = []
            order = ((1, kT_t[p]), (0, qT_t[p])) if k_first else \
                ((0, qT_t[p]), (1, kT_t[p]))
            for qk, dst in order:
                woff = qk * CPC + p * 128
                for ib in range(NIB):
                    cell = {}

                    def stepA(cell=cell, woff=woff, ib=ib):
                        cell["ps"] = psum.tile([128, IB], f32, tag="qkvps", name="qkps")
                        for kt in range(4):
                            nc.tensor.matmul(
                                cell["ps"],
                                lhsT=wqkv_sb[:, kt, woff:woff + 128],
                                rhs=xT_sb[:, kt, ib * IB:(ib + 1) * IB],
                                start=(kt == 0), stop=False)

                    def stepB(cell=cell, woff=woff, ib=ib, dst=dst):
                        for kt in range(4, NKT):
                            nc.tensor.matmul(
                                cell["ps"],
                                lhsT=wqkv_sb[:, kt, woff:woff + 128],
                                rhs=xT_sb[:, kt, ib * IB:(ib + 1) * IB],
                                start=False, stop=(kt == NKT - 1))
                        nc.vector.tensor_copy(
                            out=dst[:, ib * IB:(ib + 1) * IB], in_=cell["ps"])

                    steps += [stepA, stepB]
            return steps

        def qk_steps_fine(p):
            """Like qk_steps but 4 two-matmul bursts per (qk, ib) chunk so
            the PE load per attention iteration stays smooth."""
            steps = []
            for qk, dst in ((0, qT_t[p]), (1, kT_t[p])):
                woff = qk * CPC + p * 128
                for ib in range(NIB):
                    cell = {}

                    def mk(kk, cell=cell, woff=woff, ib=ib, dst=dst):
                        def step():
                            if kk == 0:
                                cell["ps"] = psum.tile([128, IB], f32,
                                                       tag="qkvps", name="qkps")
                            for kt in (2 * kk, 2 * kk + 1):
                                nc.tensor.matmul(
                                    cell["ps"],
                                    lhsT=wqkv_sb[:, kt, woff:woff + 128],
                                    rhs=xT_sb[:, kt, ib * IB:(ib + 1) * IB],
                                    start=(kt == 0), stop=(kt == NKT - 1))
                            if kk == 3:
                                nc.vector.tensor_copy(
                                    out=dst[:, ib * IB:(ib + 1) * IB],
                                    in_=cell["ps"])
                        return step

                    steps += [mk(kk) for kk in range(4)]
            return steps

        def emit_v(jt):
            ps = psum.tile([128, CPC], f32, tag="qkvps")
            for kt in range(NKT):
                nc.tensor.matmul(
                    ps,
                    lhsT=xT_sb[:, kt, jt * 128:(jt + 1) * 128],
                    rhs=wqkv_sb[:, kt, 2 * CPC:3 * CPC],
                    start=(kt == 0), stop=(kt == NKT - 1))
            nc.vector.tensor_copy(out=v_sb[:, jt, :], in_=ps)

        oT_all = {}

        def outproj_steps(ib):
            steps = []
            for isub in range(4):
                for nh in range(2):
                    def step(isub=isub, nh=nh, ib=ib):
                        ops = psum.tile([128, 512], f32, tag="qkvps")
                        for p in range(NPAIR):
                            nc.tensor.matmul(
                                ops,
                                lhsT=oT_all[(p, ib)][:, isub * 128:(isub + 1) * 128],
                                rhs=wout_sb[:, p, nh * 512:(nh + 1) * 512],
                                start=(p == 0), stop=(p == NPAIR - 1))
                        ost = outp.tile([128, 512], bf16, tag="ost")
                        nc.vector.tensor_copy(out=ost, in_=ops)
                        nc.sync.dma_start(
                            out_d[ib * IB + isub * 128: ib * IB + (isub + 1) * 128,
                                  nh * 512:(nh + 1) * 512], ost)
                    steps.append(step)
            return steps

        # ---- attention unit ----
        # P^T for a unit lives in two half tiles (j-tiles 0-7 / 8-15), each
        # [128, 2*NJH, IB] bf16 with planes indexed 2*jt_local + head.
        # Denominator merges run as batched strided-AP adds once their
        # sources are consumed by PV.
        def emit_S(p, ib, jt, sAB, lo_t, hi_t):
            isl_ = slice(ib * IB, (ib + 1) * IB)
            jsl = slice(jt * 128, (jt + 1) * 128)
            t, j = (lo_t, jt) if jt < NJH else (hi_t, jt - NJH)
            nc.tensor.matmul(
                sAB[:, 0:IB],
                lhsT=kT_t[p][0:64, jsl],
                rhs=qT_t[p][0:64, isl_],
                start=True, stop=True, tile_position=(0, 0))
            nc.tensor.matmul(
                sAB[:, IB:2 * IB],
                lhsT=kT_t[p][64:128, jsl],
                rhs=qT_t[p][64:128, isl_],
                start=True, stop=True, tile_position=(64, 0))
            nc.scalar.activation(
                t[:, 2 * j:2 * j + 2, :].rearrange("p a b -> p (a b)"),
                sAB, Exp, scale=SCALE)

        def unit(p, ib, extras, handoff=None, next_info=None, tail_in=None,
                 tail_pos=2):
            isl = slice(ib * IB, (ib + 1) * IB)
            if handoff is None:
                lo = ppool.tile([128, 2 * NJH, IB], bf16, tag="ptlo")
            else:
                lo = handoff
            hi = ppool1.tile([128, 2 * NJH, IB], bf16, tag="pthi")
            oT_ps = psum.tile([128, IB], f32, tag="oT")

            def pthalf(jt):
                return (lo, jt) if jt < NJH else (hi, jt - NJH)

            def emit_pv(jt):
                t, j = pthalf(jt)
                st = (jt == 0)
                sp = (jt == NJT - 1)
                nc.tensor.matmul(
                    oT_ps[0:64, :],
                    lhsT=v_sb[:, jt, (2 * p) * DH:(2 * p + 1) * DH],
                    rhs=t[:, 2 * j, :],
                    start=st, stop=sp, tile_position=(0, 0))
                nc.tensor.matmul(
                    oT_ps[64:128, :],
                    lhsT=v_sb[:, jt, (2 * p + 1) * DH:(2 * p + 2) * DH],
                    rhs=t[:, 2 * j + 1, :],
                    start=st, stop=sp, tile_position=(0, 64))

            # batched balanced merge tree over pair-planes (both heads at
            # once) for the denominators: strided-AP adds run two merges in
            # one DVE instruction.
            def pl(t, a, n=2):
                return t[:, a:a + n, :].rearrange("p a b -> p (a b)")

            def mergeL1(t, half):
                u = t.rearrange("p (g x) i -> p g (x i)", g=4)
                sl = slice(0, 2) if half == 0 else slice(2, 4)
                nc.vector.tensor_tensor(
                    u[:, sl, 0:1024], u[:, sl, 0:1024], u[:, sl, 1024:2048],
                    add)

            def mergeL2(t):
                u = t.rearrange("p (g x) i -> p g (x i)", g=2)
                nc.vector.tensor_tensor(
                    u[:, :, 0:1024], u[:, :, 0:1024], u[:, :, 2048:3072], add)

            def mergeL3(t):
                u = t.rearrange("p x i -> p (x i)")
                nc.vector.tensor_tensor(
                    u[:, 0:1024], u[:, 0:1024], u[:, 4096:5120], add)

            msched = {5: [lambda: mergeL1(lo, 0)],
                      9: [lambda: mergeL1(lo, 1)],
                      10: [lambda: mergeL2(lo)],
                      11: [lambda: mergeL3(lo)]}
            final = next_info is None
            if final:
                # Final unit: build the hi-half denominator sums
                # OUT-OF-PLACE into dead lo planes (pair-planes 2..7 are
                # dead after the lo tree at iter 11), paced right behind
                # the exps, so only (jt14,jt15) work remains after the
                # last exp instead of the whole destructive hi tree.
                lf = lo.rearrange("p x i -> p (x i)")
                hf = hi.rearrange("p (g x) i -> p g (x i)", g=4)

                def tt(dst, a, b):
                    nc.vector.tensor_tensor(dst, a, b, add)

                msched[13] = [lambda: tt(lf[:, 2048:4096],
                                         hf[:, 0:2, 0:1024],
                                         hf[:, 0:2, 1024:2048])]   # s89,s1011
                msched[14] = [lambda: tt(lf[:, 5120:6144],
                                         lf[:, 2048:3072],
                                         lf[:, 3072:4096]),        # A=s89+s1011
                              lambda: tt(lf[:, 4096:5120],
                                         hf[:, 2, 0:1024],
                                         hf[:, 2, 1024:2048])]     # s1213
                msched[15] = [lambda: tt(lf[:, 6144:7168],
                                         lf[:, 5120:6144],
                                         lf[:, 4096:5120]),        # B=A+s1213
                              lambda: tt(lf[:, 7168:8192],
                                         lf[:, 0:1024],
                                         lf[:, 6144:7168])]        # rest=lo0+B
            else:
                msched[13] = [lambda: mergeL1(hi, 0)]
            extras = dict(extras)
            nxt_lo = None
            for jt in range(NJT):
                if jt > 0 or handoff is None:
                    sAB = psum.tile([128, 2 * IB], f32, tag="sAB")
                    emit_S(p, ib, jt, sAB, lo, hi)
                if jt >= 2:
                    emit_pv(jt - 2)
                if jt == NJT - 1 and next_info is not None:
                    # emit the next unit's first S before this iter's extras
                    # so the exp stream never gaps at the unit boundary
                    np_, nib = next_info
                    nxt_lo = ppool.tile([128, 2 * NJH, IB], bf16, tag="ptlo",
                                        name="ptlo_h")
                    sAB_h = psum.tile([128, 2 * IB], f32, tag="sAB",
                                      name="sAB_h")
                    emit_S(np_, nib, 0, sAB_h, nxt_lo, None)
                for fn in extras.pop(jt, ()):
                    fn()
                if jt == tail_pos and tail_in is not None:
                    tail_in()
                for fn in msched.get(jt, ()):
                    fn()
            emit_pv(NJT - 2)
            emit_pv(NJT - 1)
            acc = spool.tile([128, 2, IB], bf16, tag="acc")
            if final:
                # only (jt14,jt15) sums remain after the last exp
                tt(lf[:, 2048:3072], hf[:, 3, 0:1024], hf[:, 3, 1024:2048])
                tt(acc.rearrange("p a b -> p (a b)"),
                   lf[:, 7168:8192], lf[:, 2048:3072])
            else:
                mergeL1(hi, 1)
                mergeL2(hi)
                mergeL3(hi)
                nc.vector.tensor_tensor(
                    acc.rearrange("p a b -> p (a b)"), pl(lo, 0), pl(hi, 0),
                    add)

            # The denominator tail (ones-matmul partition reduce, reciprocal
            # off PSUM, gpsimd broadcast, normalize) is returned as a closure
            # and injected into the NEXT unit's j-loop: by then the DVE merge
            # chain has drained, so the ones-matmul doesn't stall the
            # in-order PE queue at the unit boundary.
            def tail(final=False):
                dnr = spool.tile([1, 2, IB], f32, tag="dnr")
                if final:
                    # the last unit's sAB buffers are idle by now; using
                    # them keeps the qkvps rotation free for the warm-up
                    # output-projection chains emitted around this tail.
                    big = psum.tile([128, 2 * IB], f32, tag="sAB",
                                    name="dnps_f")
                    dps_h = [big[0:1, 0:IB], big[0:1, IB:2 * IB]]
                else:
                    dps_h = None
                for h in range(2):
                    if final:
                        dps = dps_h[h]
                    else:
                        dps = psum.tile([1, IB], f32, tag="qkvps", name="dnps")
                    nc.tensor.matmul(dps, lhsT=ones_sb[:, 0:1],
                                     rhs=acc[:, h, :], start=True, stop=True)
                    nc.vector.reciprocal_approx_fast(dnr[:, h, :], dps)
                dn = spool.tile([128, 2, IB], f32, tag="dn")
                nc.gpsimd.partition_broadcast(
                    dn.rearrange("p a b -> p (a b)"),
                    dnr.rearrange("p a b -> p (a b)"), channels=128)
                oT_sb = opool.tile([128, IB], bf16, tag="oTsb")
                nc.vector.tensor_tensor(
                    oT_sb[0:64, :], oT_ps[0:64, :], dn[0:64, 0, :], mult)
                nc.vector.tensor_tensor(
                    oT_sb[64:128, :], oT_ps[64:128, :], dn[64:128, 1, :], mult)
                oT_all[(p, ib)] = oT_sb
            return nxt_lo, tail

        # ---- prologue: just k i-block 0 and q i-block 0 of pair 0; the
        # remaining pair-0 chunks and all V projections run as extras
        # inside unit (0,0) so the scalar engine starts exp'ing early.
        p0 = qk_steps(0, k_first=True)
        # interleave the k/q chains: q's kt0-3 matmuls stream while k's
        # kt4-7 DMA chunks are still arriving
        for st in (p0[0], p0[8], p0[1], p0[9]):
            st()
        k0_rest = p0[2:8]
        q0_rest = p0[10:16]

        # ---- main sweep: pair-outer / i-block-inner ----
        # extras injected per unit: v projections + pair-0 leftovers in
        # unit (0,0); next pair's projection bursts otherwise (p<3); the
        # previous i-block's output projection for p==3.
        seq = [(p, ib) for p in range(NPAIR) for ib in range(NIB)]
        handoff = None
        pending_tail = None
        qk_cache = {}
        for i, (p, ib) in enumerate(seq):
            extras = {}
            if p == 0 and ib == 0:
                for jt in range(NJT):
                    extras.setdefault(min(jt + 1, NJT - 1), []).append(
                        lambda jt=jt: emit_v(jt))
                for pos, st in zip((1, 2, 3, 4, 5, 6), k0_rest):
                    extras.setdefault(pos, []).append(st)
                for pos, st in zip((7, 8, 9, 10, 11, 12), q0_rest):
                    extras.setdefault(pos, []).append(st)
            elif p == 0:
                if 1 not in qk_cache:
                    qk_cache[1] = qk_steps_fine(1)
                nxt = qk_cache[1]
                for pos, st in zip((1, 2, 3, 4, 6, 7, 8, 9, 11, 12, 13, 14),
                                   nxt[12 * (ib - 1): 12 * (ib - 1) + 12]):
                    extras.setdefault(pos, []).append(st)
            elif p + 1 < NPAIR:
                # pairs 2/3 spread over all four units of the previous pair
                # (8 steps at alternating iters) so per-iteration PE work
                # stays below the exp cadence and ACT never starves.
                if p + 1 not in qk_cache:
                    qk_cache[p + 1] = qk_steps_fine(p + 1)
                nxt = qk_cache[p + 1]
                for pos, st in zip((1, 3, 5, 7, 9, 11, 13, 15),
                                   nxt[8 * ib: 8 * ib + 8]):
                    extras.setdefault(pos, []).append(st)
            elif ib >= 1:
                for pos, st in zip((3, 5, 7, 9, 11, 13, 14, 15),
                                   outproj_steps(ib - 1)):
                    extras.setdefault(pos, []).append(st)
            nxt_info = seq[i + 1] if i + 1 < len(seq) else None
            # tail at iter 5 keeps the qkvps psum rotation clear of the
            # qk-projection bursts at iters 1-4; outproj units are clear
            # at iter 2 (their extras start at 3).
            # tail position avoids iterations where the qkvps psum rotation
            # is fully held by projection chains: iter 5 for the 12-step
            # scheme (pair 1), iter 8 for the 8-step scheme (pairs 2/3),
            # iter 2 for outproj units (their extras start at 3).
            if p == 0:
                tpos = 5 if ib > 0 else 2
            elif p < 3:
                tpos = 8
            else:
                tpos = 2
            handoff, tail = unit(p, ib, extras, handoff=handoff,
                                 next_info=nxt_info, tail_in=pending_tail,
                                 tail_pos=tpos)
            pending_tail = tail

        # final i-block output projection: the first two steps' pair-0..2
        # accumulation chains are emitted before the last denominator tail
        # so the PE stays busy (and warm) while the DVE merge chain drains.
        fsteps = outproj_steps(NIB - 1)
        held = []
        for s in range(3):
            isub, nh = divmod(s, 2)
            tag = "qkvps" if s < 2 else "oT"
            ops = psum.tile([128, 512], f32, tag=tag)
            for p in range(NPAIR - 1):
                nc.tensor.matmul(
                    ops,
                    lhsT=oT_all[(p, NIB - 1)][:, isub * 128:(isub + 1) * 128],
                    rhs=wout_sb[:, p, nh * 512:(nh + 1) * 512],
                    start=(p == 0), stop=False)
            held.append((ops, isub, nh))
        pending_tail(final=True)
        for ops, isub, nh in held:
            nc.tensor.matmul(
                ops,
                lhsT=oT_all[(NPAIR - 1, NIB - 1)][:, isub * 128:(isub + 1) * 128],
                rhs=wout_sb[:, NPAIR - 1, nh * 512:(nh + 1) * 512],
                start=False, stop=True)
            ost = outp.tile([128, 512], bf16, tag="ost")
            nc.vector.tensor_copy(out=ost, in_=ops)
            nc.sync.dma_start(
                out_d[(NIB - 1) * IB + isub * 128:
                      (NIB - 1) * IB + (isub + 1) * 128,
                      nh * 512:(nh + 1) * 512], ost)
        for st in fsteps[3:]:
            st()


def _prep_inputs(x, w_qkv, w_out):
    bf = ml_dtypes.bfloat16
    in_maps = []
    for c in range(8):
        b, hh = c // 2, c % 2
        xT = np.ascontiguousarray(x[b].T).astype(bf)
        q = w_qkv[:, hh * CPC:(hh + 1) * CPC]
        k = w_qkv[:, DIM + hh * CPC: DIM + (hh + 1) * CPC]
        v = w_qkv[:, 2 * DIM + hh * CPC: 2 * DIM + (hh + 1) * CPC]
        wqkv = np.ascontiguousarray(np.concatenate([q, k, v], axis=1)).astype(bf)
        wout = np.ascontiguousarray(w_out[hh * CPC:(hh + 1) * CPC, :]).astype(bf)
        in_maps.append({"xT": xT, "wqkv": wqkv, "wout": wout})
    return in_maps


def _run(x, w_qkv, w_out, b_out, trace=False):
    from concourse import bass_utils
    if "nc" not in _cache:
        _cache["nc"] = _build()
    nc = _cache["nc"]
    in_maps = _prep_inputs(x, w_qkv, w_out)
    res = bass_utils.run_bass_kernel_spmd(
        nc, in_maps, core_ids=list(range(8)), trace=trace)
    partials = [r["out"] for r in res.results]
    out = np.empty((B, N, DIM), dtype=np.float32)
    for b in range(B):
        out[b] = (partials[2 * b].astype(np.float32) +
                  partials[2 * b + 1].astype(np.float32) + b_out.astype(np.float32))
    return out, res


def kernel(x, w_qkv, w_out, b_out):
    x = np.asarray(x, dtype=np.float32)
    w_qkv = np.asarray(w_qkv, dtype=np.float32)
    w_out = np.asarray(w_out, dtype=np.float32)
    b_out = np.asarray(b_out, dtype=np.float32)
    out, _ = _run(x, w_qkv, w_out, b_out, trace=False)
    return out


# revision 63
# speedup vs baseline: 1.0103x; 1.0103x over previous
"""Multi-head attention (B=4, N=2048, DIM=1024, H=16, DH=64) on 8 trn2 cores.

Sharding: core c handles batch c//2 and head-half c%2 (8 heads).  Each core
computes qkv projection for its heads, attention, and a partial output
projection; the host sums the two partials per batch and adds the bias.
No cross-core collectives needed.

Layout strategy (zero on-device transposes):
  - host supplies x[b] pre-transposed (xT: [DIM, N]) in bf16
  - qT/kT computed as [d, n] ("transposed") via out = W^T @ x^T matmuls
  - S^T tiles [j=128, i=512] from row-packed matmuls (d=64 contraction,
    2 heads concurrently in PE row groups 0-63 / 64-127)
  - exp via ACT (scale folded), PSUM -> SBUF bf16 (P^T tiles)
  - PV: O^T[d, i] += V[j, d]^T-matmul, col-packed pairs (PE col groups),
    delayed two j-iterations behind S so the PE never waits on the
    current exp (breaks the per-iteration S->exp->PV semaphore stall)
  - denominators: batched strided-AP DVE add-tree over j-tiles, ones-matmul
    partition reduce, reciprocal straight off PSUM, gpsimd broadcast; the
    whole tail is deferred into the NEXT unit's j-loop so it never stalls
    the in-order PE queue at a unit boundary
  - first unit software-floods the V projection as per-iteration extras so
    exp starts ~27us earlier; input DMAs are issued across SP+ACT queues
    ordered by first use
  - output projection consumes O^T tiles directly as lhsT; the final
    i-block's first steps pre-accumulate pairs 0-2 to keep the PE warm
    while the last denominator drains
"""

import numpy as np
import ml_dtypes

B, N, DIM = 4, 2048, 1024
HEADS, DH = 16, 64
SCALE = DIM ** (-0.5)
HPC = 8              # heads per core
NPAIR = HPC // 2     # 4 head pairs
CPC = HPC * DH       # 512 channels per core
IB = 512             # i-block (query cols per attention unit)
NIB = N // IB        # 4
NJT = N // 128       # 16 j-tiles
NKT = DIM // 128     # 8 contraction tiles for projections

_cache = {}


def _build():
    import concourse.bacc as bacc
    import concourse.mybir as mybir
    import concourse.tile as tile

    f32 = mybir.dt.float32
    bf16 = mybir.dt.bfloat16

    nc = bacc.Bacc("TRN2", target_bir_lowering=False, debug=False,
                   enable_asserts=False, num_devices=8)

    xT_d = nc.dram_tensor("xT", (DIM, N), bf16, kind="ExternalInput").ap()
    wqkv_d = nc.dram_tensor("wqkv", (DIM, 3 * CPC), bf16, kind="ExternalInput").ap()
    wout_d = nc.dram_tensor("wout", (CPC, DIM), bf16, kind="ExternalInput").ap()
    out_d = nc.dram_tensor("out", (N, DIM), bf16, kind="ExternalOutput").ap()

    with tile.TileContext(nc) as tc:
        _body(nc, tc, mybir, xT_d, wqkv_d, wout_d, out_d)

    nc.compile()
    return nc


def _body(nc, tc, mybir, xT_d, wqkv_d, wout_d, out_d):
    import concourse.bass_isa as bass_isa
    from contextlib import ExitStack

    f32 = mybir.dt.float32
    bf16 = mybir.dt.bfloat16
    Exp = mybir.ActivationFunctionType.Exp
    mult = mybir.AluOpType.mult
    add = mybir.AluOpType.add
    NJH = NJT // 2   # j-tiles per half (8)

    ctx = ExitStack()
    with ctx:
        wpool = ctx.enter_context(tc.tile_pool(name="weights", bufs=1))
        qkv_pool = ctx.enter_context(tc.tile_pool(name="qkv", bufs=1))
        ppool = ctx.enter_context(tc.tile_pool(name="ptiles", bufs=2))
        ppool1 = ctx.enter_context(tc.tile_pool(name="ptiles1", bufs=1))
        spool = ctx.enter_context(tc.tile_pool(name="small", bufs=2))
        outp = ctx.enter_context(tc.tile_pool(name="outstage", bufs=3))
        opool = ctx.enter_context(tc.tile_pool(name="oT", bufs=16))
        psum = ctx.enter_context(tc.tile_pool(name="psum", bufs=2, space="PSUM"))

        # ---- weights + xT load, ordered by first use: k-cols + xT i-block 0
        # feed the prologue, then q-cols, v-cols, remaining xT i-blocks ----
        wqkv_sb = wpool.tile([128, NKT, 3 * CPC], bf16)
        wqkv_r = wqkv_d.rearrange("(ko p) c -> p ko c", p=128)
        xT_sb = wpool.tile([128, NKT, N], bf16)
        xT_r = xT_d.rearrange("(ko p) n -> p ko n", p=128)
        # one contiguous [0:640] chunk per k-tile covers all q columns plus
        # pair-0 k columns with descriptor-efficient 1.25KB rows
        for kt in range(NKT):
            nc.sync.dma_start(wqkv_sb[:, kt, 0:CPC + 128],
                              wqkv_r[:, kt, 0:CPC + 128])
            nc.scalar.dma_start(xT_sb[:, kt, 0:IB], xT_r[:, kt, 0:IB])
        nc.sync.dma_start(wqkv_sb[:, :, CPC + 128:2 * CPC],
                          wqkv_r[:, :, CPC + 128:2 * CPC])
        nc.scalar.dma_start(wqkv_sb[:, :, 2 * CPC:3 * CPC],
                            wqkv_r[:, :, 2 * CPC:3 * CPC])
        for ib in range(1, NIB):
            nc.scalar.dma_start(xT_sb[:, :, ib * IB:(ib + 1) * IB],
                                xT_r[:, :, ib * IB:(ib + 1) * IB])
        wout_sb = wpool.tile([128, NPAIR, DIM], bf16)
        nc.scalar.dma_start(wout_sb, wout_d.rearrange("(po p) n -> p po n", p=128))

        ones_sb = wpool.tile([128, 1], bf16)
        nc.gpsimd.memset(ones_sb, 1.0)

        # per-pair q/k tiles (separate tiles => clean dependency tracking
        # when later pairs' projections interleave into attention units)
        qT_t = [qkv_pool.tile([128, N], bf16, tag=f"qT{p}", name=f"qT{p}") for p in range(NPAIR)]
        kT_t = [qkv_pool.tile([128, N], bf16, tag=f"kT{p}", name=f"kT{p}") for p in range(NPAIR)]
        v_sb = qkv_pool.tile([128, NJT, CPC], bf16)

        # ---- emit helpers ----
        def qk_steps(p, k_first=False):
            """Projection of qT/kT for pair p as a list of small PE bursts."""
            steps = []
            order = ((1, kT_t[p]), (0, qT_t[p])) if k_first else \
                ((0, qT_t[p]), (1, kT_t[p]))
            for qk, dst in order:
                woff = qk * CPC + p * 128
                for ib in range(NIB):
                    cell = {}

                    def stepA(cell=cell, woff=woff, ib=ib):
                        cell["ps"] = psum.tile([128, IB], f32, tag="qkvps", name="qkps")
                        for kt in range(4):
                            nc.tensor.matmul(
                                cell["ps"],
                                lhsT=wqkv_sb[:, kt, woff:woff + 128],
                                rhs=xT_sb[:, kt, ib * IB:(ib + 1) * IB],
                                start=(kt == 0), stop=False)

                    def stepB(cell=cell, woff=woff, ib=ib, dst=dst):
                        for kt in range(4, NKT):
                            nc.tensor.matmul(
                                cell["ps"],
                                lhsT=wqkv_sb[:, kt, woff:woff + 128],
                                rhs=xT_sb[:, kt, ib * IB:(ib + 1) * IB],
                                start=False, stop=(kt == NKT - 1))
                        nc.vector.tensor_copy(
                            out=dst[:, ib * IB:(ib + 1) * IB], in_=cell["ps"])

                    steps += [stepA, stepB]
            return steps

        def qk_steps_fine(p):
            """Like qk_steps but 4 two-matmul bursts per (qk, ib) chunk so
            the PE load per attention iteration stays smooth."""
            steps = []
            for qk, dst in ((0, qT_t[p]), (1, kT_t[p])):
                woff = qk * CPC + p * 128
                for ib in range(NIB):
                    cell = {}

                    def mk(kk, cell=cell, woff=woff, ib=ib, dst=dst):
                        def step():
                            if kk == 0:
                                cell["ps"] = psum.tile([128, IB], f32,
                                                       tag="qkvps", name="qkps")
                            for kt in (2 * kk, 2 * kk + 1):
                                nc.tensor.matmul(
                                    cell["ps"],
                                    lhsT=wqkv_sb[:, kt, woff:woff + 128],
                                    rhs=xT_sb[:, kt, ib * IB:(ib + 1) * IB],
                                    start=(kt == 0), stop=(kt == NKT - 1))
                            if kk == 3:
                                nc.vector.tensor_copy(
                                    out=dst[:, ib * IB:(ib + 1) * IB],
                                    in_=cell["ps"])
                        return step

                    steps += [mk(kk) for kk in range(4)]
            return steps

        def emit_v(jt):
            ps = psum.tile([128, CPC], f32, tag="qkvps")
            for kt in range(NKT):
                nc.tensor.matmul(
                    ps,
                    lhsT=xT_sb[:, kt, jt * 128:(jt + 1) * 128],
                    rhs=wqkv_sb[:, kt, 2 * CPC:3 * CPC],
                    start=(kt == 0), stop=(kt == NKT - 1))
            nc.vector.tensor_copy(out=v_sb[:, jt, :], in_=ps)

        oT_all = {}

        def outproj_steps(ib):
            steps = []
            for isub in range(4):
                for nh in range(2):
                    def step(isub=isub, nh=nh, ib=ib):
                        ops = psum.tile([128, 512], f32, tag="qkvps")
                        for p in range(NPAIR):
                            nc.tensor.matmul(
                                ops,
                                lhsT=oT_all[(p, ib)][:, isub * 128:(isub + 1) * 128],
                                rhs=wout_sb[:, p, nh * 512:(nh + 1) * 512],
                                start=(p == 0), stop=(p == NPAIR - 1))
                        ost = outp.tile([128, 512], bf16, tag="ost")
                        nc.vector.tensor_copy(out=ost, in_=ops)
                        nc.sync.dma_start(
                            out_d[ib * IB + isub * 128: ib * IB + (isub + 1) * 128,
                                  nh * 512:(nh + 1) * 512], ost)
                    steps.append(step)
            return steps

        # ---- attention unit ----
        # P^T for a unit lives in two half tiles (j-tiles 0-7 / 8-15), each
        # [128, 2*NJH, IB] bf16 with planes indexed 2*jt_local + head.
        # Denominator merges run as batched strided-AP adds once their
        # sources are consumed by PV.
        def emit_S(p, ib, jt, sAB, lo_t, hi_t):
            isl_ = slice(ib * IB, (ib + 1) * IB)
            jsl = slice(jt * 128, (jt + 1) * 128)
            t, j = (lo_t, jt) if jt < NJH else (hi_t, jt - NJH)
            nc.tensor.matmul(
                sAB[:, 0:IB],
                lhsT=kT_t[p][0:64, jsl],
                rhs=qT_t[p][0:64, isl_],
                start=True, stop=True, tile_position=(0, 0))
            nc.tensor.matmul(
                sAB[:, IB:2 * IB],
                lhsT=kT_t[p][64:128, jsl],
                rhs=qT_t[p][64:128, isl_],
                start=True, stop=True, tile_position=(64, 0))
            nc.scalar.activation(
                t[:, 2 * j:2 * j + 2, :].rearrange("p a b -> p (a b)"),
                sAB, Exp, scale=SCALE)

        def unit(p, ib, extras, handoff=None, next_info=None, tail_in=None,
                 tail_pos=2):
            isl = slice(ib * IB, (ib + 1) * IB)
            if handoff is None:
                lo = ppool.tile([128, 2 * NJH, IB], bf16, tag="ptlo")
            else:
                lo = handoff
            hi = ppool1.tile([128, 2 * NJH, IB], bf16, tag="pthi")
            oT_ps = psum.tile([128, IB], f32, tag="oT")

            def pthalf(jt):
                return (lo, jt) if jt < NJH else (hi, jt - NJH)

            def emit_pv(jt):
                t, j = pthalf(jt)
                st = (jt == 0)
                sp = (jt == NJT - 1)
                nc.tensor.matmul(
                    oT_ps[0:64, :],
                    lhsT=v_sb[:, jt, (2 * p) * DH:(2 * p + 1) * DH],
                    rhs=t[:, 2 * j, :],
                    start=st, stop=sp, tile_position=(0, 0))
                nc.tensor.matmul(
                    oT_ps[64:128, :],
                    lhsT=v_sb[:, jt, (2 * p + 1) * DH:(2 * p + 2) * DH],
                    rhs=t[:, 2 * j + 1, :],
                    start=st, stop=sp, tile_position=(0, 64))

            # batched balanced merge tree over pair-planes (both heads at
            # once) for the denominators: strided-AP adds run two merges in
            # one DVE instruction.
            def pl(t, a, n=2):
                return t[:, a:a + n, :].rearrange("p a b -> p (a b)")

            def mergeL1(t, half):
                u = t.rearrange("p (g x) i -> p g (x i)", g=4)
                sl = slice(0, 2) if half == 0 else slice(2, 4)
                nc.vector.tensor_tensor(
                    u[:, sl, 0:1024], u[:, sl, 0:1024], u[:, sl, 1024:2048],
                    add)

            def mergeL2(t):
                u = t.rearrange("p (g x) i -> p g (x i)", g=2)
                nc.vector.tensor_tensor(
                    u[:, :, 0:1024], u[:, :, 0:1024], u[:, :, 2048:3072], add)

            def mergeL3(t):
                u = t.rearrange("p x i -> p (x i)")
                nc.vector.tensor_tensor(
                    u[:, 0:1024], u[:, 0:1024], u[:, 4096:5120], add)

            msched = {5: [lambda: mergeL1(lo, 0)],
                      9: [lambda: mergeL1(lo, 1)],
                      10: [lambda: mergeL2(lo)],
                      11: [lambda: mergeL3(lo)]}
            final = next_info is None
            if final:
                # Final unit: build the hi-half denominator sums
                # OUT-OF-PLACE into dead lo planes (pair-planes 2..7 are
                # dead after the lo tree at iter 11), paced right behind
                # the exps, so only (jt14,jt15) work remains after the
                # last exp instead of the whole destructive hi tree.
                lf = lo.rearrange("p x i -> p (x i)")
                hf = hi.rearrange("p (g x) i -> p g (x i)", g=4)

                def tt(dst, a, b):
                    nc.vector.tensor_tensor(dst, a, b, add)

                msched[13] = [lambda: tt(lf[:, 2048:4096],
                                         hf[:, 0:2, 0:1024],
                                         hf[:, 0:2, 1024:2048])]   # s89,s1011
                msched[14] = [lambda: tt(lf[:, 5120:6144],
                                         lf[:, 2048:3072],
                                         lf[:, 3072:4096]),        # A=s89+s1011
                              lambda: tt(lf[:, 4096:5120],
                                         hf[:, 2, 0:1024],
                                         hf[:, 2, 1024:2048])]     # s1213
                msched[15] = [lambda: tt(lf[:, 6144:7168],
                                         lf[:, 5120:6144],
                                         lf[:, 4096:5120]),        # B=A+s1213
                              lambda: tt(lf[:, 7168:8192],
                                         lf[:, 0:1024],
                                         lf[:, 6144:7168])]        # rest=lo0+B
            else:
                msched[13] = [lambda: mergeL1(hi, 0)]
            extras = dict(extras)
            nxt_lo = None
            for jt in range(NJT):
                if jt > 0 or handoff is None:
                    sAB = psum.tile([128, 2 * IB], f32, tag="sAB")
                    emit_S(p, ib, jt, sAB, lo, hi)
                if jt >= 2:
                    emit_pv(jt - 2)
                if jt == NJT - 1 and next_info is not None:
                    # emit the next unit's first S before this iter's extras
                    # so the exp stream never gaps at the unit boundary
                    np_, nib = next_info
                    nxt_lo = ppool.tile([128, 2 * NJH, IB], bf16, tag="ptlo",
                                        name="ptlo_h")
                    sAB_h = psum.tile([128, 2 * IB], f32, tag="sAB",
                                      name="sAB_h")
                    emit_S(np_, nib, 0, sAB_h, nxt_lo, None)
                for fn in extras.pop(jt, ()):
                    fn()
                if jt == tail_pos and tail_in is not None:
                    tail_in()
                for fn in msched.get(jt, ()):
                    fn()
            emit_pv(NJT - 2)
            emit_pv(NJT - 1)
            acc = spool.tile([128, 2, IB], bf16, tag="acc")
            if final:
                # only (jt14,jt15) sums remain after the last exp
                tt(lf[:, 2048:3072], hf[:, 3, 0:1024], hf[:, 3, 1024:2048])
                tt(acc.rearrange("p a b -> p (a b)"),
                   lf[:, 7168:8192], lf[:, 2048:3072])
            else:
                mergeL1(hi, 1)
                mergeL2(hi)
                mergeL3(hi)
                nc.vector.tensor_tensor(
                    acc.rearrange("p a b -> p (a b)"), pl(lo, 0), pl(hi, 0),
                    add)

            # The denominator tail (ones-matmul partition reduce, reciprocal
            # off PSUM, gpsimd broadcast, normalize) is returned as a closure
            # and injected into the NEXT unit's j-loop: by then the DVE merge
            # chain has drained, so the ones-matmul doesn't stall the
            # in-order PE queue at the unit boundary.
            def tail(final=False):
                dnr = spool.tile([1, 2, IB], f32, tag="dnr")
                if final:
                    # the last unit's sAB buffers are idle by now; using
                    # them keeps the qkvps rotation free for the warm-up
                    # output-projection chains emitted around this tail.
                    big = psum.tile([128, 2 * IB], f32, tag="sAB",
                                    name="dnps_f")
                    dps_h = [big[0:1, 0:IB], big[0:1, IB:2 * IB]]
                else:
                    dps_h = None
                for h in range(2):
                    if final:
                        dps = dps_h[h]
                    else:
                        dps = psum.tile([1, IB], f32, tag="qkvps", name="dnps")
                    nc.tensor.matmul(dps, lhsT=ones_sb[:, 0:1],
                                     rhs=acc[:, h, :], start=True, stop=True)
                    nc.vector.reciprocal_approx_fast(dnr[:, h, :], dps)
                dn = spool.tile([128, 2, IB], f32, tag="dn")
                nc.gpsimd.partition_broadcast(
                    dn.rearrange("p a b -> p (a b)"),
                    dnr.rearrange("p a b -> p (a b)"), channels=128)
                oT_sb = opool.tile([128, IB], bf16, tag="oTsb")
                nc.vector.tensor_tensor(
                    oT_sb[0:64, :], oT_ps[0:64, :], dn[0:64, 0, :], mult)
                nc.vector.tensor_tensor(
                    oT_sb[64:128, :], oT_ps[64:128, :], dn[64:128, 1, :], mult)
                oT_all[(p, ib)] = oT_sb
            return nxt_lo, tail

        # ---- prologue: just k i-block 0 and q i-block 0 of pair 0; the
        # remaining pair-0 chunks and all V projections run as extras
        # inside unit (0,0) so the scalar engine starts exp'ing early.
        p0 = qk_steps(0, k_first=True)
        # interleave the k/q chains: q's kt0-3 matmuls stream while k's
        # kt4-7 DMA chunks are still arriving
        for st in (p0[0], p0[8], p0[1], p0[9]):
            st()
        k0_rest = p0[2:8]
        q0_rest = p0[10:16]

        # ---- main sweep: pair-outer / i-block-inner ----
        # extras injected per unit: v projections + pair-0 leftovers in
        # unit (0,0); next pair's projection bursts otherwise (p<3); the
        # previous i-block's output projection for p==3.
        seq = [(p, ib) for p in range(NPAIR) for ib in range(NIB)]
        handoff = None
        pending_tail = None
        qk_cache = {}
        for i, (p, ib) in enumerate(seq):
            extras = {}
            if p == 0 and ib == 0:
                for jt in range(NJT):
                    extras.setdefault(min(jt + 1, NJT - 1), []).append(
                        lambda jt=jt: emit_v(jt))
                for pos, st in zip((1, 2, 3, 4, 5, 6), k0_rest):
                    extras.setdefault(pos, []).append(st)
                for pos, st in zip((7, 8, 9, 10, 11, 12), q0_rest):
                    extras.setdefault(pos, []).append(st)
            elif p == 0:
                if 1 not in qk_cache:
                    qk_cache[1] = qk_steps_fine(1)
                nxt = qk_cache[1]
                for pos, st in zip((1, 2, 3, 4, 6, 7, 8, 9, 11, 12, 13, 14),
                                   nxt[12 * (ib - 1): 12 * (ib - 1) + 12]):
                    extras.setdefault(pos, []).append(st)
            elif p + 1 < NPAIR:
                # pairs 2/3 spread over all four units of the previous pair
                # (8 steps at alternating iters) so per-iteration PE work
                # stays below the exp cadence and ACT never starves.
                if p + 1 not in qk_cache:
                    qk_cache[p + 1] = qk_steps_fine(p + 1)
                nxt = qk_cache[p + 1]
                for pos, st in zip((1, 3, 5, 7, 9, 11, 13, 15),
                                   nxt[8 * ib: 8 * ib + 8]):
                    extras.setdefault(pos, []).append(st)
            elif ib >= 1:
                # start at 6: oT(3, ib-1)'s normalize (gated by the previous
                # unit's merge chain via the deferred tail) lands ~iter 5;
                # earlier slots would block the PE queue on it. The end
                # bunching is absorbed because the handoff-S already fed the
                # next unit's exp stream.
                for pos, st in zip((6, 8, 10, 12, 13, 14, 15, 15),
                                   outproj_steps(ib - 1)):
                    extras.setdefault(pos, []).append(st)
            nxt_info = seq[i + 1] if i + 1 < len(seq) else None
            # tail at iter 5 keeps the qkvps psum rotation clear of the
            # qk-projection bursts at iters 1-4; outproj units are clear
            # at iter 2 (their extras start at 3).
            # tail position avoids iterations where the qkvps psum rotation
            # is fully held by projection chains: iter 5 for the 12-step
            # scheme (pair 1), iter 8 for the 8-step scheme (pairs 2/3),
            # iter 2 for outproj units (their extras start at 3).
            if p == 0:
                tpos = 5 if ib > 0 else 2
            elif p < 3:
                tpos = 8
            else:
                tpos = 2
            handoff, tail = unit(p, ib, extras, handoff=handoff,
                                 next_info=nxt_info, tail_in=pending_tail,
                                 tail_pos=tpos)
            pending_tail = tail

        # final i-block output projection: the first two steps' pair-0..2
        # accumulation chains are emitted before the last denominator tail
        # so the PE stays busy (and warm) while the DVE merge chain drains.
        fsteps = outproj_steps(NIB - 1)
        held = []
        for s in range(3):
            isub, nh = divmod(s, 2)
            tag = "qkvps" if s < 2 else "oT"
            ops = psum.tile([128, 512], f32, tag=tag)
            for p in range(NPAIR - 1):
                nc.tensor.matmul(
                    ops,
                    lhsT=oT_all[(p, NIB - 1)][:, isub * 128:(isub + 1) * 128],
                    rhs=wout_sb[:, p, nh * 512:(nh + 1) * 512],
                    start=(p == 0), stop=False)
            held.append((ops, isub, nh))
        pending_tail(final=True)
        for ops, isub, nh in held:
            nc.tensor.matmul(
                ops,
                lhsT=oT_all[(NPAIR - 1, NIB - 1)][:, isub * 128:(isub + 1) * 128],
                rhs=wout_sb[:, NPAIR - 1, nh * 512:(nh + 1) * 512],
                start=False, stop=True)
            ost = outp.tile([128, 512], bf16, tag="ost")
            nc.vector.tensor_copy(out=ost, in_=ops)
            nc.sync.dma_start(
                out_d[(NIB - 1) * IB + isub * 128:
                      (NIB - 1) * IB + (isub + 1) * 128,
                      nh * 512:(nh + 1) * 512], ost)
        for st in fsteps[3:]:
            st()


def _prep_inputs(x, w_qkv, w_out):
    bf = ml_dtypes.bfloat16
    in_maps = []
    for c in range(8):
        b, hh = c // 2, c % 2
        xT = np.ascontiguousarray(x[b].T).astype(bf)
        q = w_qkv[:, hh * CPC:(hh + 1) * CPC]
        k = w_qkv[:, DIM + hh * CPC: DIM + (hh + 1) * CPC]
        v = w_qkv[:, 2 * DIM + hh * CPC: 2 * DIM + (hh + 1) * CPC]
        wqkv = np.ascontiguousarray(np.concatenate([q, k, v], axis=1)).astype(bf)
        wout = np.ascontiguousarray(w_out[hh * CPC:(hh + 1) * CPC, :]).astype(bf)
        in_maps.append({"xT": xT, "wqkv": wqkv, "wout": wout})
    return in_maps


def _run(x, w_qkv, w_out, b_out, trace=False):
    from concourse import bass_utils
    if "nc" not in _cache:
        _cache["nc"] = _build()
    nc = _cache["nc"]
    in_maps = _prep_inputs(x, w_qkv, w_out)
    res = bass_utils.run_bass_kernel_spmd(
        nc, in_maps, core_ids=list(range(8)), trace=trace)
    partials = [r["out"] for r in res.results]
    out = np.empty((B, N, DIM), dtype=np.float32)
    for b in range(B):
        out[b] = (partials[2 * b].astype(np.float32) +
                  partials[2 * b + 1].astype(np.float32) + b_out.astype(np.float32))
    return out, res


def kernel(x, w_qkv, w_out, b_out):
    x = np.asarray(x, dtype=np.float32)
    w_qkv = np.asarray(w_qkv, dtype=np.float32)
    w_out = np.asarray(w_out, dtype=np.float32)
    b_out = np.asarray(b_out, dtype=np.float32)
    out, _ = _run(x, w_qkv, w_out, b_out, trace=False)
    return out
